# revision 8
# baseline (speedup 1.0000x reference)
"""AffNetR TRN2 kernel v4: out[u,i] = ((max_h cos(Z[h,u,:], X[i,:])) + 1) / 2, ^beta.

Sharding: data-parallel over users (U=8192) across 8 NeuronCores; X replicated.
Each core computes a [1024, 8192] slice of the output.

Normalization is folded host-side into bf16 inputs. The cross-head max uses
max(a,b) = (a+b)/2 + |a-b|/2 on sum/diff stationaries (za01 = (z0n+z1n)/4,
zd01 = (z0n-z1n)/4, same for heads 2,3). Per 512-col tile PSUM holds
[s01|s23] (2 banks, pool ps) and [d01|d23] (2 banks, pool pd):
  ACT: t = Abs([d01|d23]) -> SBUF bf16, one fs1024 op
  DVE: m = [s01|s23] + t  -> SBUF bf16, one fs1024 op (single PSUM operand)
giving m = [max(c0,c1)|max(c2,c3)] with c_h = cos_h/2. The final cross-pair
max + 0.5 affine runs on the host (numpy) over the DMA'd bf16 pair rows.

Matmuls are emitted pairwise (same stationary for 2 consecutive fs512 MMs)
so the PE streams without a weights reload between them. Every tile with
it % BPERIOD == BPHASE is a "B-tile": ACT additionally evacuates the s-pair
via Abs(s + 1) = s + 1 (bias keeps the argument positive; same ACT function
=> no activation-table reload) and DVE does a cheap bf16 2x SBUF add; the
host subtracts the extra 1 for those columns. This balances DVE vs ACT.
"""

import numpy as np
import ml_dtypes

import concourse.bass as bass
import concourse.mybir as mybir
import concourse.tile as tile
from concourse.bass_utils import run_bass_kernel_spmd

F32 = mybir.dt.float32
BF16 = mybir.dt.bfloat16
BF16_NP = ml_dtypes.bfloat16

H = 4
U = 8192
E = 128
I = 8192
NCORES = 8
USH = U // NCORES          # 1024 users per core
UT = USH // 128            # 8 u-tiles
IT = I // 512              # 16 i-tiles of 512
ZCOLS = 4 * 128            # per-ut stationary pack: za01|zd01|za23|zd23
EPS = 1e-6

# ---- tunables -------------------------------------------------------------
BPERIOD = 16               # every BPERIOD-th i-tile is a B-tile (ACT-heavy)
BPHASE = 13                # which tile in the period (avoid blk boundary 15)
# ---------------------------------------------------------------------------

_cache = {}


def _is_b(it):
    return it % BPERIOD == BPHASE


def _legalize_waits(nc, max_waits=1):
    """Hoist excess sem waits onto same-engine NoOps (1-wait ISA structs)."""
    cnt = 0
    for f in nc.m.functions:
        for blk in f.blocks:
            insts = blk.instructions
            out = []
            changed = False
            for inst in insts:
                si = inst.sync_info
                waits = list(si.on_wait) if si is not None and si.on_wait else []
                if len(waits) > max_waits and inst.engine is not None:
                    keep = waits[-max_waits:]
                    for w in waits[:-max_waits]:
                        nop = mybir.InstNoOp(name=f"wlg-{cnt}", ins=[], outs=[])
                        cnt += 1
                        nop.engine = inst.engine
                        nop.sync_info = mybir.SyncInfo(on_wait=[w], on_update=[])
                        out.append(nop)
                    upd = list(si.on_update) if si.on_update else []
                    inst.sync_info = mybir.SyncInfo(on_wait=keep, on_update=upd)
                    changed = True
                out.append(inst)
            if changed:
                blk.instructions = out
    return cnt


def _build(legalize=True):
    nc = bass.Bass()
    xs_d = nc.dram_tensor("xs", [E, I], BF16, kind="ExternalInput")
    zp_d = nc.dram_tensor("zp", [E, UT * ZCOLS], BF16, kind="ExternalInput")
    out_d = nc.dram_tensor("out", [USH, 2 * I], BF16, kind="ExternalOutput")
    out_v = out_d[:].rearrange("(uo p) i -> p uo i", p=128)

    S = mybir.ActivationFunctionType
    A = mybir.AluOpType

    with tile.TileContext(nc) as tc:
        with (
            tc.tile_pool(name="cst", bufs=1) as cst,
            tc.tile_pool(name="tb", bufs=3) as tb,
            tc.tile_pool(name="sb", bufs=2) as sbp,
            tc.tile_pool(name="rows", bufs=2) as rows,
            tc.tile_pool(name="ps", bufs=2, space="PSUM") as ps,
            tc.tile_pool(name="pd", bufs=2, space="PSUM") as pd,
        ):
            zp_sb = cst.tile([E, UT * ZCOLS], BF16, tag="zp_sb", name="zp_sb")
            xs_sb = cst.tile([E, I], BF16, tag="xs_sb", name="xs_sb")
            for u in range(UT):
                s = slice(u * ZCOLS, (u + 1) * ZCOLS)
                nc.sync.dma_start(zp_sb[:, s], zp_d[:, s])
            for c in range(I // 512):
                s = slice(c * 512, (c + 1) * 512)
                nc.sync.dma_start(xs_sb[:, s], xs_d[:, s])

            one = cst.tile([128, 1], F32, tag="one", name="one")
            nc.vector.memset(one, 1.0)

            for ut in range(UT):
                zb = ut * ZCOLS
                za01 = zp_sb[:, zb + 0 * 128 : zb + 1 * 128]
                zd01 = zp_sb[:, zb + 1 * 128 : zb + 2 * 128]
                za23 = zp_sb[:, zb + 2 * 128 : zb + 3 * 128]
                zd23 = zp_sb[:, zb + 3 * 128 : zb + 4 * 128]
                for blk in range(IT // 4):
                    mrow = rows.tile([128, 4096], BF16, tag="mrow", name="mrow")
                    for jp in range(2):  # two tile-pairs per block
                        it0 = blk * 4 + jp * 2
                        xv0 = xs_sb[:, it0 * 512 : (it0 + 1) * 512]
                        xv1 = xs_sb[:, (it0 + 1) * 512 : (it0 + 2) * 512]
                        gs0 = ps.tile([128, 1024], F32, tag="gs", name="gs0")
                        gd0 = pd.tile([128, 1024], F32, tag="gd", name="gd0")
                        gs1 = ps.tile([128, 1024], F32, tag="gs", name="gs1")
                        gd1 = pd.tile([128, 1024], F32, tag="gd", name="gd1")
                        # pairwise: each stationary feeds 2 back-to-back MMs
                        nc.tensor.matmul(gs0[:, 0:512], za01, xv0, start=True, stop=True)
                        nc.tensor.matmul(gs1[:, 0:512], za01, xv1, start=True, stop=True)
                        nc.tensor.matmul(gs0[:, 512:1024], za23, xv0, start=True, stop=True)
                        nc.tensor.matmul(gs1[:, 512:1024], za23, xv1, start=True, stop=True)
                        nc.tensor.matmul(gd0[:, 0:512], zd01, xv0, start=True, stop=True)
                        nc.tensor.matmul(gd1[:, 0:512], zd01, xv1, start=True, stop=True)
                        nc.tensor.matmul(gd0[:, 512:1024], zd23, xv0, start=True, stop=True)
                        nc.tensor.matmul(gd1[:, 512:1024], zd23, xv1, start=True, stop=True)

                        for k, (gs, gd) in enumerate(((gs0, gd0), (gs1, gd1))):
                            it = it0 + k
                            j = it % 4
                            mseg = mrow[:, j * 1024 : (j + 1) * 1024]
                            t = tb.tile([128, 1024], BF16, tag="t", name="t")
                            nc.scalar.activation(t, gd, S.Abs)
                            if _is_b(it):
                                # ACT also evacs s-pair: |s+1| = s+1 (s>=-0.5)
                                e = sbp.tile([128, 1024], BF16, tag="e", name="e")
                                nc.scalar.activation(e, gs, S.Abs, bias=one, scale=1.0)
                                nc.vector.tensor_tensor(mseg, e, t, A.add)
                            else:
                                nc.vector.tensor_tensor(mseg, gs, t, A.add)

                    nc.gpsimd.dma_start(
                        out_v[:, ut, blk * 4096 : (blk + 1) * 4096], mrow
                    )

    if legalize:
        _legalize_waits(nc)
    return nc


def _prep_inputs(X, Z):
    X = np.asarray(X, dtype=np.float32)
    Z = np.asarray(Z, dtype=np.float32)
    xn = np.linalg.norm(X, axis=1) + EPS                    # [I]
    xs = np.ascontiguousarray((X / xn[:, None]).T)          # [128, I] unit rows
    xs = xs.astype(BF16_NP)

    zn = np.linalg.norm(Z, axis=2) + EPS                    # [H, U]
    Zs = Z / zn[:, :, None]                                 # [H, U, 128] unit
    in_maps = []
    for c in range(NCORES):
        zc = Zs[:, c * USH : (c + 1) * USH, :]              # [4, 1024, 128]
        zp = np.empty((E, UT * ZCOLS), dtype=np.float32)
        for ut in range(UT):
            us = slice(ut * 128, (ut + 1) * 128)
            z0 = zc[0, us].T                                # [128e, 128u]
            z1 = zc[1, us].T
            z2 = zc[2, us].T
            z3 = zc[3, us].T
            zb = ut * ZCOLS
            zp[:, zb + 0 * 128 : zb + 1 * 128] = (z0 + z1) * 0.25
            zp[:, zb + 1 * 128 : zb + 2 * 128] = (z0 - z1) * 0.25
            zp[:, zb + 2 * 128 : zb + 3 * 128] = (z2 + z3) * 0.25
            zp[:, zb + 3 * 128 : zb + 4 * 128] = (z2 - z3) * 0.25
        in_maps.append({"xs": xs, "zp": zp.astype(BF16_NP)})
    return in_maps


def _host_final(m, out_slice):
    """m: [USH, 2*I] bf16 interleaved [m01|m23] per 512; writes f32 out."""
    m4 = m.reshape(USH, I // 512, 2, 512).astype(np.float32)
    np.maximum(m4[:, :, 0, :], m4[:, :, 1, :], out=out_slice.reshape(USH, I // 512, 512))
    # B-tiles carried an extra +1 from the Abs bias
    for it in range(IT):
        if _is_b(it):
            out_slice[:, it * 512 : (it + 1) * 512] -= 1.0


def kernel(X, Z, beta):
    in_maps = _prep_inputs(X, Z)
    if "nc" not in _cache:
        _cache["nc"] = _build()
    res = run_bass_kernel_spmd(_cache["nc"], in_maps, list(range(NCORES))).results

    out = np.empty((U, I), dtype=np.float32)
    for c in range(NCORES):
        _host_final(res[c]["out"], out[c * USH : (c + 1) * USH])
    out += 0.5

    b = float(np.asarray(beta))
    if b != 1.0:
        out = np.power(out, b).astype(np.float32)
    return out
